# revision 19
# baseline (speedup 1.0000x reference)
"""Multi-head attention (Vaswani) on Trainium2, head-parallel across 8 NeuronCores.

Problem shapes (hardcoded):
  h:   [B=2, G=2048, D=128] f32
  W_Q/W_K/W_V: [H=8, D=128, K=16] f32
  out: [B=2, H=8, G=2048, V=16] f32  = softmax(0.25 * (h@Wq) @ (h@Wk)^T) @ (h@Wv)

Sharding: one head per core (8 heads / 8 cores). Each core receives the full h
plus its head's weight slices, computes [B, G, V]; host stacks on the head axis.

Per-core plan (v4). The baseline (v2, kernel_v2_baseline.py, 68452 ns) was
lock-step saturated on BOTH the Scalar engine (exp stream, ~64us busy) and
PE (compat+AV fp32r streams + staging, ~64us). v4 attacks both:

  1. exp offload: a Schraudolph fast-exp (one DVE tensor_scalar mult-add
     writing int16 bits that ARE the bf16 exp value) handles `n_dve` of the
     64 (batch, q-slice, chunk-pair) units; ACT does true exp for the rest.
     Softmax cancels the systematic exp error; measured end-to-end l2 with
     ALL units on DVE is 6.8e-3, so n_dve~14 adds ~3e-3.
  2. compat row-packing: chunk pairs run CONCURRENTLY in PE row groups 0/32
     (tile_position), halving the compat stream. Needs kT/qT replicated at
     partitions 0-15 and 32-47, which the projection matmuls produce for
     free via replicated weight columns (wq2/wk2 [128,64], zero-padded).
  3. bf16 matmul operand tiles (hT/qT2/kT2/vp/at): same PE cycles, but FWL
     weight loads, cheaper staging copies, and no f32r "produced rounded"
     constraint on the bitcast exp trick. Accuracy stack end to end:
     l2 = 8.25e-3 on HW vs the 2e-2 gate.
  4. DVE diet (DVE is the exp-offload budget): 4 transposes / 4 v-proj
     chunks funneled through ONE psum bank each -> single wide copies;
     normalize uses one [128,68] transpose target, one strided reciprocal
     [128,4] and one 0-stride-broadcast tensor_tensor multiply per slice.

Per (batch, 512-wide q-slice): 8 chunk-pair units: packed compat MMs into a
[128,1024] PSUM tile (2 banks), one 1024-wide exp (ACT or DVE), two AV MMs
accumulating oT[17,512] (ones column in v' accumulates the softmax
denominator). Slice ends: oT -> sbuf, 4 PE transposes into one [128,68]
psum tile, reciprocal+broadcast-mult, one out-DMA per slice. Input staging
for the next batch is popped a few ops per unit into the main loop
(emission order is dependency order for Tile — see the npre comment).

cfg fp8_av=True switches ACT units to fp8e4 attn/v' with paired DoubleRow
AV matmuls: correct on HW (l2 1.54e-2, exp bias -3.5 to dodge e4m3
overflow->NaN) but measured SLOWER than the bf16 path — off by default.

Interleaved A/B slope bench under identical contention (compare.py):
v4 ~107 us/rep vs v2 ~177 us/rep -> ~1.65x faster.
"""

import numpy as np

B, G, D = 2, 2048, 128
H, K, V = 8, 16, 16
N_CORES = 8
P = 128
GT = G // P          # 16 key chunks of 128
QB = 512             # q-slice width (one fp32 PSUM bank)
NSL = G // QB        # 4 q-slices per batch
NPAIR = GT // 2      # 8 chunk pairs per slice
VP1 = V + 1          # v' width (ones column appended)
VPW = VP1            # v' chunk stride

DEFAULT_CFG = {
    "n_dve": 18,       # of the 64 units, how many exp on DVE (Schraudolph)
    "fp8_av": False,   # ACT units: fp8 attn + v', paired DoubleRow AV MMs
    "at_bufs": 6,      # attnT sbuf buffers
    "pc_bufs": 2,      # compat psum buffers
    "reps": 1,         # repeat whole kernel body (for HW slope timing)
    "pops": 3,         # staged ops popped per unit
}

_CACHE = {}


def _build(cfg_key):
    cfg = dict(DEFAULT_CFG)
    cfg.update(dict(cfg_key))
    import concourse.bacc as bacc
    import concourse.mybir as mybir
    from concourse.tile import TileContext
    from concourse.masks import make_identity

    f32 = mybir.dt.float32
    bf16 = mybir.dt.bfloat16
    fp8 = mybir.dt.float8e4
    i16 = mybir.dt.int16
    EXP = mybir.ActivationFunctionType.Exp
    MULT = mybir.AluOpType.mult
    ADD = mybir.AluOpType.add
    DR = mybir.MatmulPerfMode.DoubleRow
    fp8_av = bool(cfg["fp8_av"])
    # With fp8 attn weights the exp must be shifted down so e^z fits e4m3's
    # [2^-9, 448] range: z = 0.25*x - 2 (logits are within +-8.5; the shift
    # cancels between softmax numerator and denominator).
    EBIAS = -3.5 if fp8_av else 0.0
    # bf16-space Schraudolph constants (exp(0.25*x + EBIAS) via bit trick):
    # bits16 = rint((0.25*x + EBIAS) * 2^7/ln2 + 127*2^7), as bf16.
    A16 = float(0.25 * (2 ** 7) / np.log(2.0))
    B16 = float(127 * 2 ** 7 + EBIAS * (2 ** 7) / np.log(2.0))

    n_dve = int(cfg["n_dve"])
    NU = B * NSL * NPAIR  # 64 units per rep

    def unit_on_dve(uid):
        return (uid + 1) * n_dve // NU > uid * n_dve // NU

    nc = bacc.Bacc("TRN2", debug=False, enable_asserts=False,
                   target_bir_lowering=False)
    h_d = nc.dram_tensor("h", [B, G, D], f32, kind="ExternalInput").ap()
    wq_d = nc.dram_tensor("wq", [D, K], f32, kind="ExternalInput").ap()
    wk_d = nc.dram_tensor("wk", [D, K], f32, kind="ExternalInput").ap()
    wv_d = nc.dram_tensor("wv", [D, V], f32, kind="ExternalInput").ap()
    out_d = nc.dram_tensor("out", [B, G, V], f32, kind="ExternalOutput").ap()

    with TileContext(nc) as tc:
        with tc.tile_pool(name="const", bufs=1) as cpool, \
             tc.tile_pool(name="sc", bufs=2, space="PSUM") as scpool, \
             tc.tile_pool(name="pc", bufs=cfg["pc_bufs"],
                          space="PSUM") as pcpool, \
             tc.tile_pool(name="po", bufs=2, space="PSUM") as popool, \
             tc.tile_pool(name="att", bufs=cfg["at_bufs"]) as apool:
            ident = cpool.tile([P, P], f32)
            make_identity(nc, ident)
            warm = cpool.tile([P, 1], f32)
            nc.scalar.activation(warm, ident[:, 0:1], EXP)
            biast = cpool.tile([P, 1], f32)
            nc.vector.memset(biast, EBIAS)
            w_sb = cpool.tile([D, 3 * K], f32)
            wq2 = cpool.tile([D, 64], bf16)
            wk2 = cpool.tile([D, 64], bf16)
            wv_r = cpool.tile([D, V], bf16)

            def load_w():
                nc.sync.dma_start(w_sb[:, 0:K], wq_d)
                nc.sync.dma_start(w_sb[:, K:2 * K], wk_d)
                nc.sync.dma_start(w_sb[:, 2 * K:3 * K], wv_d)

            def build_w():
                nc.vector.memset(wq2, 0.0)
                nc.vector.memset(wk2, 0.0)
                for g in (0, 32):
                    nc.vector.tensor_copy(wq2[:, g:g + K], w_sb[:, 0:K])
                    nc.vector.tensor_copy(wk2[:, g:g + K], w_sb[:, K:2 * K])
                nc.vector.tensor_copy(wv_r, w_sb[:, 2 * K:3 * K])

            hA_b, hT_b, qT_b, kT_b, vp_b, v8_b, ob_b = [], [], [], [], [], [], []
            for b in range(B):
                hA_b.append(cpool.tile([P, G], f32, name=f"hA{b}"))
                hT_b.append(cpool.tile([P, G], bf16, name=f"hT{b}"))
                qT_b.append(cpool.tile([48, G], bf16, name=f"qT{b}"))
                kT_b.append(cpool.tile([48, G], bf16, name=f"kT{b}"))
                vp_b.append(cpool.tile([P, GT * VPW], bf16, name=f"vp{b}"))
                if fp8_av:
                    # v' chunk pairs for DoubleRow: pair p at 64p, chunks at
                    # +0 and +32 (Ko step 32 B), ones column at +16/+48.
                    v8_b.append(cpool.tile([P, NPAIR * 64], fp8,
                                           name=f"v8{b}"))
                else:
                    v8_b.append(None)
                ob_b.append(cpool.tile([P, GT * V], f32, name=f"ob{b}"))

            def init_vp():
                for b in range(B):
                    nc.vector.memset(
                        vp_b[b].rearrange("p (t w) -> p t w", w=VPW)[:, :, V:],
                        1.0)
                    if fp8_av:
                        nc.vector.memset(
                            v8_b[b].rearrange(
                                "p (x w) -> p x w", w=32)[:, :, V:V + 1],
                            1.0)

            def phase1_ops(b):
                """Input staging for batch b, in dependency order; popped a
                few per unit inside the previous batch's main loop."""
                hA, hT, qT2, kT2, vp, vp8 = (hA_b[b], hT_b[b], qT_b[b],
                                             kT_b[b], vp_b[b], v8_b[b])

                def dmaq(qq):
                    nc.sync.dma_start(
                        hA[:, qq * 4 * P:(qq + 1) * 4 * P].rearrange(
                            "p (t d) -> p t d", t=4),
                        h_d[b, qq * 4 * P:(qq + 1) * 4 * P, :].rearrange(
                            "(t p) d -> p t d", p=P))

                def trq(qq):
                    # 4 chunk transposes into one psum bank, single copy out
                    pt = scpool.tile([P, QB], f32, tag="s", name="pt")
                    for j in range(4):
                        t = 4 * qq + j
                        nc.tensor.transpose(pt[:, j * P:(j + 1) * P],
                                            hA[:, t * P:(t + 1) * P], ident)
                    nc.vector.tensor_copy(
                        hT[:, qq * 4 * P:(qq + 1) * 4 * P], pt)

                def proj(qb, w2, dst):
                    sl = slice(qb * QB, (qb + 1) * QB)
                    pq = scpool.tile([P, QB], f32, tag="s", name="pq")
                    nc.tensor.matmul(pq[0:64, :], w2, hT[:, sl],
                                     start=True, stop=True)
                    nc.vector.tensor_copy(dst[0:48, sl], pq[0:48, :])

                def vprojq(qq):
                    # 4 chunks' v' into one psum tile, then one strided copy
                    pvv = scpool.tile([P, QB], f32, tag="s", name="pvv")
                    for j in range(4):
                        t = 4 * qq + j
                        nc.tensor.matmul(pvv[:, j * V:(j + 1) * V],
                                         hT[:, t * P:(t + 1) * P],
                                         wv_r, start=True, stop=True)
                    src = pvv[:, 0:4 * V].rearrange("p (j v) -> p j v", v=V)
                    nc.vector.tensor_copy(
                        vp.rearrange("p (t w) -> p t w", w=VPW)
                        [:, 4 * qq:4 * qq + 4, 0:V], src)
                    if fp8_av:
                        nc.vector.tensor_copy(
                            v8.rearrange("p (x w) -> p x w", w=32)
                            [:, 4 * qq:4 * qq + 4, 0:V], src)

                v8 = vp8
                ops = [lambda qq=qq: dmaq(qq) for qq in range(4)]
                for qq in range(4):
                    ops.append(lambda qq=qq: trq(qq))
                    ops.append(lambda qq=qq: proj(qq, wk2, kT2))
                    ops.append(lambda qq=qq: vprojq(qq))
                    ops.append(lambda qq=qq: proj(qq, wq2, qT2))
                return ops

            units = [(rr, bb) for rr in range(cfg["reps"])
                     for bb in range(B)]
            first = phase1_ops(units[0][1])
            first = (first[0:1] + [load_w] + first[1:4] + [build_w, init_vp]
                     + first[4:])
            # prefix: inits + h DMAs + quarter 0 (4 ops). The first q-slice's
            # units consume chunks in pair order; the in-loop pops (3 per
            # unit, after each unit's MMs) must emit quarter q's trq/projk/
            # vprojq (pending idx 4q-4..4q-2) before unit 2q+2's MMs (3
            # pops per earlier unit: idx < 6q+6) — ample slack for q<=3.
            npre = 11
            for op in first[:npre]:
                op()
            pending = first[npre:]
            uid = 0
            for ui, (rep, b) in enumerate(units):
                qT2, kT2, vp, ob_all = (qT_b[b], kT_b[b], vp_b[b], ob_b[b])
                if ui + 1 < len(units):
                    pending = pending + phase1_ops(units[ui + 1][1])

                vp8 = v8_b[b]
                for s in range(NSL):
                    q0 = s * QB
                    oT = popool.tile([VP1, QB], f32, tag="oT", name="oT")
                    for p in range(NPAIR):
                        c0, c1 = 2 * p, 2 * p + 1
                        cps = pcpool.tile([P, 2 * QB], f32, tag="c",
                                          name="cps")
                        nc.tensor.matmul(
                            cps[:, 0:QB],
                            kT2[0:K, c0 * P:(c0 + 1) * P],
                            qT2[0:K, q0:q0 + QB],
                            start=True, stop=True, tile_position=(0, 0))
                        nc.tensor.matmul(
                            cps[:, QB:2 * QB],
                            kT2[32:32 + K, c1 * P:(c1 + 1) * P],
                            qT2[32:32 + K, q0:q0 + QB],
                            start=True, stop=True, tile_position=(32, 0))
                        dve_unit = unit_on_dve(uid % NU)
                        if dve_unit:
                            at = apool.tile([P, 2 * QB], bf16, tag="at",
                                            name="at")
                            nc.vector.tensor_scalar(
                                at.bitcast(i16), cps, A16, B16, MULT, ADD)
                            nc.tensor.matmul(
                                oT, vp[:, c0 * VPW:c0 * VPW + VP1],
                                at[:, 0:QB],
                                start=(p == 0), stop=False,
                                skip_group_check=True)
                            nc.tensor.matmul(
                                oT, vp[:, c1 * VPW:c1 * VPW + VP1],
                                at[:, QB:2 * QB],
                                start=False, stop=(p == NPAIR - 1),
                                skip_group_check=True)
                        elif fp8_av:
                            at8 = apool.tile([P, 2 * QB], fp8, tag="at8",
                                             name="at8")
                            nc.scalar.activation(at8, cps, EXP,
                                                 scale=0.25, bias=biast)
                            nc.tensor.matmul(
                                oT,
                                vp8[:, p * 64:(p + 1) * 64].rearrange(
                                    "p (k w) -> p k w", k=2)[:, :, 0:VP1],
                                at8.rearrange("p (k n) -> p k n", k=2),
                                start=(p == 0), stop=(p == NPAIR - 1),
                                perf_mode=DR, skip_group_check=True)
                        else:
                            at = apool.tile([P, 2 * QB], bf16, tag="at",
                                            name="at")
                            nc.scalar.activation(at, cps, EXP, scale=0.25)
                            nc.tensor.matmul(
                                oT, vp[:, c0 * VPW:c0 * VPW + VP1],
                                at[:, 0:QB],
                                start=(p == 0), stop=False,
                                skip_group_check=True)
                            nc.tensor.matmul(
                                oT, vp[:, c1 * VPW:c1 * VPW + VP1],
                                at[:, QB:2 * QB],
                                start=False, stop=(p == NPAIR - 1),
                                skip_group_check=True)
                        uid += 1
                        for _ in range(cfg["pops"]):
                            if pending:
                                pending.pop(0)()

                    # normalize this q-slice: transpose the four 128-q tiles
                    # into ONE [128, 68] psum tile, then a single reciprocal
                    # + broadcast-multiply pass
                    oT_sb = apool.tile([VP1, QB], f32, tag="oTsb",
                                       name="oT_sb")
                    nc.vector.tensor_copy(oT_sb, oT)
                    pf = scpool.tile([P, QB], f32, tag="s", name="pf")
                    for tl in range(QB // P):
                        nc.tensor.transpose(
                            pf[:, tl * VP1:(tl + 1) * VP1],
                            oT_sb[:, tl * P:(tl + 1) * P],
                            ident[:VP1, :VP1])
                    pf3 = pf[:, 0:4 * VP1].rearrange("p (t w) -> p t w",
                                                     w=VP1)
                    rcp = apool.tile([P, 4], f32, tag="rcp", name="rcp")
                    nc.vector.reciprocal(rcp.unsqueeze(2),
                                         pf3[:, :, V:V + 1])
                    nc.vector.tensor_tensor(
                        ob_all[:, 4 * s * V:4 * (s + 1) * V].rearrange(
                            "p (t v) -> p t v", v=V),
                        pf3[:, :, 0:V],
                        rcp.unsqueeze(2).broadcast_to([P, 4, V]),
                        MULT)

                    # per-slice out DMA so the store overlaps the next
                    nc.sync.dma_start(
                        out_d[b, q0:q0 + QB, :].rearrange(
                            "(t p) v -> p t v", p=P),
                        ob_all[:, (q0 // P) * V:((q0 + QB) // P) * V]
                        .rearrange("p (t v) -> p t v", t=QB // P))

                for op in pending:
                    op()
                pending = []

    nc.compile()
    return nc


def _get(cfg=None):
    cfg = cfg or {}
    key = tuple(sorted({**DEFAULT_CFG, **cfg}.items()))
    if key not in _CACHE:
        _CACHE[key] = _build(key)
    return _CACHE[key]


def _in_maps(h, W_Q, W_K, W_V):
    h = np.ascontiguousarray(np.asarray(h, dtype=np.float32))
    W_Q = np.asarray(W_Q, dtype=np.float32)
    W_K = np.asarray(W_K, dtype=np.float32)
    W_V = np.asarray(W_V, dtype=np.float32)
    return [
        {"h": h, "wq": np.ascontiguousarray(W_Q[c]),
         "wk": np.ascontiguousarray(W_K[c]),
         "wv": np.ascontiguousarray(W_V[c])}
        for c in range(N_CORES)
    ]


def kernel(h, W_Q, W_K, W_V, cfg=None, **run_kwargs):
    from concourse import bass_utils
    nc = _get(cfg)
    res = bass_utils.run_bass_kernel_spmd(
        nc, _in_maps(h, W_Q, W_K, W_V),
        core_ids=list(range(N_CORES)), **run_kwargs)
    out = np.stack([res.results[c]["out"] for c in range(N_CORES)], axis=1)
    kernel.last_results = res
    return out


# revision 23
# speedup vs baseline: 81.6927x; 81.6927x over previous
"""Multi-head attention (Vaswani) on Trainium2, head-parallel across 8 NeuronCores.

Problem shapes (hardcoded):
  h:   [B=2, G=2048, D=128] f32
  W_Q/W_K/W_V: [H=8, D=128, K=16] f32
  out: [B=2, H=8, G=2048, V=16] f32  = softmax(0.25 * (h@Wq) @ (h@Wk)^T) @ (h@Wv)

Sharding: one head per core (8 heads / 8 cores). Each core receives the full h
plus its head's weight slices, computes [B, G, V]; host stacks on the head axis.

Per-core plan (v4). The baseline (v2, kernel_v2_baseline.py, 68452 ns) was
lock-step saturated on BOTH the Scalar engine (exp stream, ~64us busy) and
PE (compat+AV fp32r streams + staging, ~64us). v4 attacks both:

  1. exp offload: a Schraudolph fast-exp (one DVE tensor_scalar mult-add
     writing int16 bits that ARE the bf16 exp value) handles `n_dve` of the
     64 (batch, q-slice, chunk-pair) units; ACT does true exp for the rest.
     Softmax cancels the systematic exp error; measured end-to-end l2 with
     ALL units on DVE is 6.8e-3, so n_dve~14 adds ~3e-3.
  2. compat row-packing: chunk pairs run CONCURRENTLY in PE row groups 0/32
     (tile_position), halving the compat stream. Needs kT/qT replicated at
     partitions 0-15 and 32-47, which the projection matmuls produce for
     free via replicated weight columns (wq2/wk2 [128,64], zero-padded).
  3. bf16 matmul operand tiles (hT/qT2/kT2/vp/at): same PE cycles, but FWL
     weight loads, cheaper staging copies, and no f32r "produced rounded"
     constraint on the bitcast exp trick. Accuracy stack end to end:
     l2 = 8.25e-3 on HW vs the 2e-2 gate.
  4. DVE diet (DVE is the exp-offload budget): 4 transposes / 4 v-proj
     chunks funneled through ONE psum bank each -> single wide copies;
     normalize uses one [128,68] transpose target, one strided reciprocal
     [128,4] and one 0-stride-broadcast tensor_tensor multiply per slice.

Per (batch, 512-wide q-slice): 8 chunk-pair units: packed compat MMs into a
[128,1024] PSUM tile (2 banks), one 1024-wide exp (ACT or DVE), two AV MMs
accumulating oT[17,512] (ones column in v' accumulates the softmax
denominator). Slice ends: oT -> sbuf, 4 PE transposes into one [128,68]
psum tile, reciprocal+broadcast-mult, one out-DMA per slice. Input staging
for the next batch is popped a few ops per unit into the main loop
(emission order is dependency order for Tile — see the npre comment).

cfg fp8_av=True switches ACT units to fp8e4 attn/v' with paired DoubleRow
AV matmuls: correct on HW (l2 1.54e-2, exp bias -3.5 to dodge e4m3
overflow->NaN) but measured SLOWER than the bf16 path — off by default.

Interleaved A/B slope bench under identical contention (compare.py):
v4 ~107 us/rep vs v2 ~177 us/rep -> ~1.65x faster.
"""

import numpy as np

B, G, D = 2, 2048, 128
H, K, V = 8, 16, 16
N_CORES = 8
P = 128
GT = G // P          # 16 key chunks of 128
QB = 512             # q-slice width (one fp32 PSUM bank)
NSL = G // QB        # 4 q-slices per batch
NPAIR = GT // 2      # 8 chunk pairs per slice
VP1 = V + 1          # v' width (ones column appended)
VPW = VP1            # v' chunk stride

DEFAULT_CFG = {
    "n_dve": 18,       # of the 64 units, how many exp on DVE (Schraudolph)
    "fp8_av": False,   # ACT units: fp8 attn + v', paired DoubleRow AV MMs
    "at_bufs": 8,      # attnT sbuf buffers (deeper exp->AV look-ahead)
    "pc_bufs": 2,      # compat psum buffers
    "reps": 1,         # repeat whole kernel body (for HW slope timing)
    "pops": 3,         # staged ops popped per unit
}

_CACHE = {}


def _build(cfg_key):
    cfg = dict(DEFAULT_CFG)
    cfg.update(dict(cfg_key))
    import concourse.bacc as bacc
    import concourse.mybir as mybir
    from concourse.tile import TileContext
    from concourse.masks import make_identity

    f32 = mybir.dt.float32
    bf16 = mybir.dt.bfloat16
    fp8 = mybir.dt.float8e4
    i16 = mybir.dt.int16
    EXP = mybir.ActivationFunctionType.Exp
    MULT = mybir.AluOpType.mult
    ADD = mybir.AluOpType.add
    DR = mybir.MatmulPerfMode.DoubleRow
    fp8_av = bool(cfg["fp8_av"])
    # With fp8 attn weights the exp must be shifted down so e^z fits e4m3's
    # [2^-9, 448] range: z = 0.25*x - 2 (logits are within +-8.5; the shift
    # cancels between softmax numerator and denominator).
    EBIAS = -3.5 if fp8_av else 0.0
    # bf16-space Schraudolph constants (exp(0.25*x + EBIAS) via bit trick):
    # bits16 = rint((0.25*x + EBIAS) * 2^7/ln2 + 127*2^7), as bf16.
    A16 = float(0.25 * (2 ** 7) / np.log(2.0))
    B16 = float(127 * 2 ** 7 + EBIAS * (2 ** 7) / np.log(2.0))

    n_dve = int(cfg["n_dve"])
    NU = B * NSL * NPAIR  # 64 units per rep

    def unit_on_dve(uid):
        return (uid + 1) * n_dve // NU > uid * n_dve // NU

    nc = bacc.Bacc("TRN2", debug=False, enable_asserts=False,
                   target_bir_lowering=False)
    h_d = nc.dram_tensor("h", [B, G, D], f32, kind="ExternalInput").ap()
    wq_d = nc.dram_tensor("wq", [D, K], f32, kind="ExternalInput").ap()
    wk_d = nc.dram_tensor("wk", [D, K], f32, kind="ExternalInput").ap()
    wv_d = nc.dram_tensor("wv", [D, V], f32, kind="ExternalInput").ap()
    out_d = nc.dram_tensor("out", [B, G, V], f32, kind="ExternalOutput").ap()

    with TileContext(nc) as tc:
        with tc.tile_pool(name="const", bufs=1) as cpool, \
             tc.tile_pool(name="sc", bufs=2, space="PSUM") as scpool, \
             tc.tile_pool(name="pc", bufs=cfg["pc_bufs"],
                          space="PSUM") as pcpool, \
             tc.tile_pool(name="po", bufs=2, space="PSUM") as popool, \
             tc.tile_pool(name="att", bufs=cfg["at_bufs"]) as apool:
            ident = cpool.tile([P, P], f32)
            make_identity(nc, ident)
            warm = cpool.tile([P, 1], f32)
            nc.scalar.activation(warm, ident[:, 0:1], EXP)
            biast = cpool.tile([P, 1], f32)
            nc.vector.memset(biast, EBIAS)
            w_sb = cpool.tile([D, 3 * K], f32)
            wq2 = cpool.tile([D, 64], bf16)
            wk2 = cpool.tile([D, 64], bf16)
            wv_r = cpool.tile([D, V], bf16)

            def load_w():
                nc.sync.dma_start(w_sb[:, 0:K], wq_d)
                nc.sync.dma_start(w_sb[:, K:2 * K], wk_d)
                nc.sync.dma_start(w_sb[:, 2 * K:3 * K], wv_d)

            def build_w():
                nc.vector.memset(wq2, 0.0)
                nc.vector.memset(wk2, 0.0)
                for g in (0, 32):
                    nc.vector.tensor_copy(wq2[:, g:g + K], w_sb[:, 0:K])
                    nc.vector.tensor_copy(wk2[:, g:g + K], w_sb[:, K:2 * K])
                nc.vector.tensor_copy(wv_r, w_sb[:, 2 * K:3 * K])

            hA_b, hT_b, qT_b, kT_b, vp_b, v8_b, ob_b = [], [], [], [], [], [], []
            for b in range(B):
                hA_b.append(cpool.tile([P, G], f32, name=f"hA{b}"))
                hT_b.append(cpool.tile([P, G], bf16, name=f"hT{b}"))
                qT_b.append(cpool.tile([48, G], bf16, name=f"qT{b}"))
                kT_b.append(cpool.tile([48, G], bf16, name=f"kT{b}"))
                vp_b.append(cpool.tile([P, GT * VPW], bf16, name=f"vp{b}"))
                if fp8_av:
                    # v' chunk pairs for DoubleRow: pair p at 64p, chunks at
                    # +0 and +32 (Ko step 32 B), ones column at +16/+48.
                    v8_b.append(cpool.tile([P, NPAIR * 64], fp8,
                                           name=f"v8{b}"))
                else:
                    v8_b.append(None)
                ob_b.append(cpool.tile([P, GT * V], f32, name=f"ob{b}"))

            def init_vp():
                for b in range(B):
                    nc.vector.memset(
                        vp_b[b].rearrange("p (t w) -> p t w", w=VPW)[:, :, V:],
                        1.0)
                    if fp8_av:
                        nc.vector.memset(
                            v8_b[b].rearrange(
                                "p (x w) -> p x w", w=32)[:, :, V:V + 1],
                            1.0)

            def phase1_ops(b):
                """Input staging for batch b, in dependency order; popped a
                few per unit inside the previous batch's main loop."""
                hA, hT, qT2, kT2, vp, vp8 = (hA_b[b], hT_b[b], qT_b[b],
                                             kT_b[b], vp_b[b], v8_b[b])

                def dmaq(qq):
                    nc.sync.dma_start(
                        hA[:, qq * 4 * P:(qq + 1) * 4 * P].rearrange(
                            "p (t d) -> p t d", t=4),
                        h_d[b, qq * 4 * P:(qq + 1) * 4 * P, :].rearrange(
                            "(t p) d -> p t d", p=P))

                def trq(qq):
                    # 4 chunk transposes into one psum bank, single copy out
                    # (f32r-bitcast transposes rejected by walrus codegen)
                    pt = scpool.tile([P, QB], f32, tag="s", name="pt")
                    for j in range(4):
                        t = 4 * qq + j
                        nc.tensor.transpose(pt[:, j * P:(j + 1) * P],
                                            hA[:, t * P:(t + 1) * P], ident)
                    nc.vector.tensor_copy(
                        hT[:, qq * 4 * P:(qq + 1) * 4 * P], pt)

                def proj(qb, w2, dst):
                    sl = slice(qb * QB, (qb + 1) * QB)
                    pq = scpool.tile([P, QB], f32, tag="s", name="pq")
                    nc.tensor.matmul(pq[0:64, :], w2, hT[:, sl],
                                     start=True, stop=True)
                    nc.vector.tensor_copy(dst[0:48, sl], pq[0:48, :])

                def vprojq(qq):
                    # 4 chunks' v' into one psum tile, then one strided copy
                    pvv = scpool.tile([P, QB], f32, tag="s", name="pvv")
                    for j in range(4):
                        t = 4 * qq + j
                        nc.tensor.matmul(pvv[:, j * V:(j + 1) * V],
                                         hT[:, t * P:(t + 1) * P],
                                         wv_r, start=True, stop=True)
                    src = pvv[:, 0:4 * V].rearrange("p (j v) -> p j v", v=V)
                    nc.vector.tensor_copy(
                        vp.rearrange("p (t w) -> p t w", w=VPW)
                        [:, 4 * qq:4 * qq + 4, 0:V], src)
                    if fp8_av:
                        nc.vector.tensor_copy(
                            v8.rearrange("p (x w) -> p x w", w=32)
                            [:, 4 * qq:4 * qq + 4, 0:V], src)

                v8 = vp8
                ops = [lambda qq=qq: dmaq(qq) for qq in range(4)]
                for qq in range(4):
                    ops.append(lambda qq=qq: trq(qq))
                    ops.append(lambda qq=qq: proj(qq, wk2, kT2))
                    # projq before vprojq: projq gates the quarter's first
                    # compat MMs, vprojq only the AV ~1us later
                    ops.append(lambda qq=qq: proj(qq, wq2, qT2))
                    ops.append(lambda qq=qq: vprojq(qq))
                return ops

            units = [(rr, bb) for rr in range(cfg["reps"])
                     for bb in range(B)]
            first = phase1_ops(units[0][1])
            first = (first[0:1] + [load_w] + first[1:4] + [build_w, init_vp]
                     + first[4:])
            # prefix: inits + h DMAs + quarter 0 (4 ops). The first q-slice's
            # units consume chunks in pair order; the in-loop pops (3 per
            # unit, after each unit's MMs) must emit quarter q's trq/projk/
            # vprojq (pending idx 4q-4..4q-2) before unit 2q+2's MMs (3
            # pops per earlier unit: idx < 6q+6) — ample slack for q<=3.
            npre = 11
            for op in first[:npre]:
                op()
            pending = first[npre:]
            uid = 0
            for ui, (rep, b) in enumerate(units):
                qT2, kT2, vp, ob_all = (qT_b[b], kT_b[b], vp_b[b], ob_b[b])
                if ui + 1 < len(units):
                    pending = pending + phase1_ops(units[ui + 1][1])

                vp8 = v8_b[b]
                for s in range(NSL):
                    q0 = s * QB
                    oT = popool.tile([VP1, QB], f32, tag="oT", name="oT")
                    for p in range(NPAIR):
                        c0, c1 = 2 * p, 2 * p + 1
                        cps = pcpool.tile([P, 2 * QB], f32, tag="c",
                                          name="cps")
                        nc.tensor.matmul(
                            cps[:, 0:QB],
                            kT2[0:K, c0 * P:(c0 + 1) * P],
                            qT2[0:K, q0:q0 + QB],
                            start=True, stop=True, tile_position=(0, 0))
                        nc.tensor.matmul(
                            cps[:, QB:2 * QB],
                            kT2[32:32 + K, c1 * P:(c1 + 1) * P],
                            qT2[32:32 + K, q0:q0 + QB],
                            start=True, stop=True, tile_position=(32, 0))
                        dve_unit = unit_on_dve(uid % NU)
                        if dve_unit:
                            at = apool.tile([P, 2 * QB], bf16, tag="at",
                                            name="at")
                            nc.vector.tensor_scalar(
                                at.bitcast(i16), cps, A16, B16, MULT, ADD)
                            nc.tensor.matmul(
                                oT, vp[:, c0 * VPW:c0 * VPW + VP1],
                                at[:, 0:QB],
                                start=(p == 0), stop=False,
                                skip_group_check=True)
                            nc.tensor.matmul(
                                oT, vp[:, c1 * VPW:c1 * VPW + VP1],
                                at[:, QB:2 * QB],
                                start=False, stop=(p == NPAIR - 1),
                                skip_group_check=True)
                        elif fp8_av:
                            at8 = apool.tile([P, 2 * QB], fp8, tag="at8",
                                             name="at8")
                            nc.scalar.activation(at8, cps, EXP,
                                                 scale=0.25, bias=biast)
                            nc.tensor.matmul(
                                oT,
                                vp8[:, p * 64:(p + 1) * 64].rearrange(
                                    "p (k w) -> p k w", k=2)[:, :, 0:VP1],
                                at8.rearrange("p (k n) -> p k n", k=2),
                                start=(p == 0), stop=(p == NPAIR - 1),
                                perf_mode=DR, skip_group_check=True)
                        else:
                            at = apool.tile([P, 2 * QB], bf16, tag="at",
                                            name="at")
                            nc.scalar.activation(at, cps, EXP, scale=0.25)
                            nc.tensor.matmul(
                                oT, vp[:, c0 * VPW:c0 * VPW + VP1],
                                at[:, 0:QB],
                                start=(p == 0), stop=False,
                                skip_group_check=True)
                            nc.tensor.matmul(
                                oT, vp[:, c1 * VPW:c1 * VPW + VP1],
                                at[:, QB:2 * QB],
                                start=False, stop=(p == NPAIR - 1),
                                skip_group_check=True)
                        uid += 1
                        for _ in range(cfg["pops"]):
                            if pending:
                                pending.pop(0)()

                    # normalize this q-slice: transpose the four 128-q tiles
                    # into ONE [128, 68] psum tile, then a single reciprocal
                    # + broadcast-multiply pass
                    oT_sb = apool.tile([VP1, QB], f32, tag="oTsb",
                                       name="oT_sb")
                    nc.vector.tensor_copy(oT_sb, oT)
                    pf = scpool.tile([P, QB], f32, tag="s", name="pf")
                    for tl in range(QB // P):
                        nc.tensor.transpose(
                            pf[:, tl * VP1:(tl + 1) * VP1],
                            oT_sb[:, tl * P:(tl + 1) * P],
                            ident[:VP1, :VP1])
                    pf3 = pf[:, 0:4 * VP1].rearrange("p (t w) -> p t w",
                                                     w=VP1)
                    rcp = apool.tile([P, 4], f32, tag="rcp", name="rcp")
                    nc.vector.reciprocal(rcp.unsqueeze(2),
                                         pf3[:, :, V:V + 1])
                    nc.vector.tensor_tensor(
                        ob_all[:, 4 * s * V:4 * (s + 1) * V].rearrange(
                            "p (t v) -> p t v", v=V),
                        pf3[:, :, 0:V],
                        rcp.unsqueeze(2).broadcast_to([P, 4, V]),
                        MULT)

                    # per-slice out DMA so the store overlaps the next
                    nc.sync.dma_start(
                        out_d[b, q0:q0 + QB, :].rearrange(
                            "(t p) v -> p t v", p=P),
                        ob_all[:, (q0 // P) * V:((q0 + QB) // P) * V]
                        .rearrange("p (t v) -> p t v", t=QB // P))

                for op in pending:
                    op()
                pending = []

    nc.compile()
    return nc


def _get(cfg=None):
    cfg = cfg or {}
    key = tuple(sorted({**DEFAULT_CFG, **cfg}.items()))
    if key not in _CACHE:
        _CACHE[key] = _build(key)
    return _CACHE[key]


def _in_maps(h, W_Q, W_K, W_V):
    h = np.ascontiguousarray(np.asarray(h, dtype=np.float32))
    W_Q = np.asarray(W_Q, dtype=np.float32)
    W_K = np.asarray(W_K, dtype=np.float32)
    W_V = np.asarray(W_V, dtype=np.float32)
    return [
        {"h": h, "wq": np.ascontiguousarray(W_Q[c]),
         "wk": np.ascontiguousarray(W_K[c]),
         "wv": np.ascontiguousarray(W_V[c])}
        for c in range(N_CORES)
    ]


def kernel(h, W_Q, W_K, W_V, cfg=None, **run_kwargs):
    from concourse import bass_utils
    nc = _get(cfg)
    res = bass_utils.run_bass_kernel_spmd(
        nc, _in_maps(h, W_Q, W_K, W_V),
        core_ids=list(range(N_CORES)), **run_kwargs)
    out = np.stack([res.results[c]["out"] for c in range(N_CORES)], axis=1)
    kernel.last_results = res
    return out
